# revision 15
# baseline (speedup 1.0000x reference)
"""Trainium2 Bass kernel for the BAN (bilinear attention network) problem.

Math (per batch b, eval mode):
    hq = emb[he_ques] @ Wq + bq                  [NQ, H]
    hk = emb[he_kg]   @ Wk + bk                  [NK, H]
    logits[g,q,k] = sum_d hq[q,d] Watt[d,g] hk[k,d]   (+ batt[g], which cancels
                                                       in the joint softmax)
    att = softmax over flattened (q,k) per (b,g)
    pooled[g,d] = sum_{q,k} hq[q,d] att[g,q,k] hk[k,d]
    out = pooled.flat @ Wout + bout;  sim = out @ glove.T;  log_softmax(sim)

Distribution: pure data parallel over batch, 8 samples per core on 8 cores.
All weights replicated. No collectives.

Matmul operands use float32r (single-pass PE mode, ~1.3e-4 matmul accuracy
vs 4x slower two-pass fp32); accumulation stays fp32 in PSUM.

Layout strategy (per core, B_loc=8):
  - emb is host-augmented with a ones column (E=301) so projection biases
    ride in the matmul; tokens gathered on-device via indirect DMA:
    X [token, 301], PE-transposed to X.T [E, token].
  - hq.T [d, tok] and hk.T [d, tok] from lhsT=W, rhs=X.T
  - hk   [tok, d] from lhsT=X.T, rhs=W
  - logits.T [k, (g,q)] = (hk.T tiles).T @ (hq.T * Watt[:,g])  -> logits are
    O(+-6), so exp() without max subtraction is safe in fp32; the joint
    softmax normalization Z_g = sum E is applied to pooled.T per sample.
  - u.T [d, (g,q)] = (hk tiles).T @ E.T;  v = u.T * hq.T (bcast over g);
    pooled.T[d, g] = reduce_q v, scaled by 1/Z_g.
  - out [8, 300] = (pooled.T as lhsT).T @ Wout tiles;  sim [8, 4000] via
    lhsT=out.T (PE transpose), rhs=glove.T;  log-softmax on [8, 4000].
  - Wout/glove.T tile DMAs are emitted up-front so they stream into SBUF
    while the attention loop runs.
"""

import sys

if "/opt/trn_rl_repo" not in sys.path:
    sys.path.insert(0, "/opt/trn_rl_repo")

import numpy as np

import concourse.bass as bass
import concourse.mybir as mybir
import concourse.tile as tile
from concourse import bacc
from concourse.bass_utils import run_bass_kernel_spmd

F32 = mybir.dt.float32
F32R = mybir.dt.float32r
I32 = mybir.dt.int32
AX = mybir.AxisListType
OP = mybir.AluOpType
AF = mybir.ActivationFunctionType

N_CORES = 8
VOCAB = 20000
E = 300          # word embedding size
EA = E + 1       # augmented with ones column (bias trick)
H = 1024         # hidden
G = 8            # heads
N_OUT = 300
N_ANS = 4000
B, NQ, NK = 64, 32, 256
BL = B // N_CORES            # 8 samples per core
TQ = BL * NQ                 # 256 q tokens per core
TK = BL * NK                 # 2048 k tokens per core
TQ_TILES = TQ // 128         # 2
TK_TILES = TK // 128         # 16
DT = H // 128                # 8 d-tiles
E_CHUNKS = (128, 128, EA - 256)   # (128, 128, 45)
N_CHUNKS = (128, 128, N_OUT - 256)
NA_CH = 8                    # sim computed in 8 chunks of 500
NA_W = N_ANS // NA_CH        # 500
NWOUT = G * DT               # 64 Wout k-tiles


def build_kernel():
    nc = bacc.Bacc("TRN2", target_bir_lowering=False, debug=False,
                   num_devices=N_CORES)

    # ---- DRAM I/O ----
    emb_d = nc.dram_tensor("emb", [VOCAB, EA], F32R, kind="ExternalInput").ap()
    idxq_d = nc.dram_tensor("idx_q", [128, TQ_TILES], I32, kind="ExternalInput").ap()
    idxk_d = nc.dram_tensor("idx_k", [128, TK_TILES], I32, kind="ExternalInput").ap()
    wq_d = nc.dram_tensor("wq", [EA, H], F32R, kind="ExternalInput").ap()
    wk_d = nc.dram_tensor("wk", [EA, H], F32R, kind="ExternalInput").ap()
    watt_d = nc.dram_tensor("watt", [128, DT, G], F32, kind="ExternalInput").ap()
    wout_d = nc.dram_tensor("wout", [G * H, N_OUT], F32R, kind="ExternalInput").ap()
    bout_d = nc.dram_tensor("bout", [BL, N_OUT], F32, kind="ExternalInput").ap()
    glovet_d = nc.dram_tensor("glovet", [N_OUT, N_ANS], F32R,
                              kind="ExternalInput").ap()
    ident_d = nc.dram_tensor("ident", [128, 128], F32R, kind="ExternalInput").ap()
    ones_d = nc.dram_tensor("ones_col", [128, 1], F32R, kind="ExternalInput").ap()
    out_d = nc.dram_tensor("out", [BL, N_ANS], F32, kind="ExternalOutput").ap()

    with tile.TileContext(nc) as tc:
        import contextlib

        with contextlib.ExitStack() as ctx:
            consts = ctx.enter_context(tc.tile_pool(name="consts", bufs=1))
            xrow_p = ctx.enter_context(tc.tile_pool(name="xrow", bufs=4))
            xkt_p = ctx.enter_context(tc.tile_pool(name="xkt", bufs=2))
            hkt_p = ctx.enter_context(tc.tile_pool(name="hkt", bufs=2))
            hk_p = ctx.enter_context(tc.tile_pool(name="hk", bufs=2))
            hqw_p = ctx.enter_context(tc.tile_pool(name="hqw", bufs=2))
            et_p = ctx.enter_context(tc.tile_pool(name="et", bufs=2))
            v_p = ctx.enter_context(tc.tile_pool(name="v", bufs=2))
            zz_p = ctx.enter_context(tc.tile_pool(name="zz", bufs=2))
            zn_p = ctx.enter_context(tc.tile_pool(name="zn", bufs=2))
            wout_p = ctx.enter_context(tc.tile_pool(name="wout", bufs=20))
            glove_p = ctx.enter_context(tc.tile_pool(name="glove", bufs=3))
            mm_p = ctx.enter_context(tc.tile_pool(name="mm", bufs=4, space="PSUM"))
            lg_p = ctx.enter_context(tc.tile_pool(name="lg", bufs=2, space="PSUM"))
            up_p = ctx.enter_context(tc.tile_pool(name="up", bufs=2, space="PSUM"))

            # ---- constants into SBUF ----
            ident = consts.tile([128, 128], F32R, tag="ident")
            nc.sync.dma_start(ident[:], ident_d)
            idxq_sb = consts.tile([128, TQ_TILES], I32, tag="idxq")
            nc.sync.dma_start(idxq_sb[:], idxq_d)
            idxk_sb = consts.tile([128, TK_TILES], I32, tag="idxk")
            nc.sync.dma_start(idxk_sb[:], idxk_d)
            wq_sb = consts.tile([128, 3, H], F32R, tag="wq")
            wk_sb = consts.tile([128, 3, H], F32R, tag="wk")
            for c, rows in enumerate(E_CHUNKS):
                nc.sync.dma_start(wq_sb[:rows, c, :], wq_d[c * 128 : c * 128 + rows])
                nc.sync.dma_start(wk_sb[:rows, c, :], wk_d[c * 128 : c * 128 + rows])
            watt_sb = consts.tile([128, DT, G], F32, tag="watt")
            nc.sync.dma_start(watt_sb[:], watt_d)
            bout_sb = consts.tile([BL, N_OUT], F32, tag="bout")
            nc.sync.dma_start(bout_sb[:], bout_d)
            ones_sb = consts.tile([128, 1], F32R, tag="ones")
            nc.sync.dma_start(ones_sb[:], ones_d)

            # ---- early-issued weight streams for phases F/G ----
            wout_tiles = []
            for t in range(NWOUT):
                wtile = wout_p.tile([128, N_OUT], F32R, tag="wout")
                nc.sync.dma_start(wtile[:], wout_d[t * 128 : (t + 1) * 128, :])
                wout_tiles.append(wtile)
            glove_tiles = []
            for a in range(NA_CH):
                gt = glove_p.tile([128, 3, NA_W], F32R, tag="glove")
                for c, rows in enumerate(N_CHUNKS):
                    nc.sync.dma_start(
                        gt[:rows, c, :],
                        glovet_d[c * 128 : c * 128 + rows,
                                 a * NA_W : (a + 1) * NA_W],
                    )
                glove_tiles.append(gt)

            def gather_transpose(idx_sb, col, dst, dst_col):
                """Gather 128 emb rows (token tile) and write transpose into
                dst[:, c, dst_col*128:...] per E-chunk c. The emb ones column
                (E index 300) lands at partition 44 of chunk 2."""
                xrow = xrow_p.tile([128, EA], F32R, tag="xrow")
                nc.gpsimd.indirect_dma_start(
                    out=xrow[:],
                    out_offset=None,
                    in_=emb_d,
                    in_offset=bass.IndirectOffsetOnAxis(
                        ap=idx_sb[:, col : col + 1], axis=0
                    ),
                )
                for c, rows in enumerate(E_CHUNKS):
                    ps = mm_p.tile([128, 512], F32R, tag="mm")
                    nc.tensor.transpose(
                        ps[:rows, :128], xrow[:, c * 128 : c * 128 + rows], ident[:]
                    )
                    nc.any.tensor_copy(
                        out=dst[:rows, c, dst_col * 128 : (dst_col + 1) * 128],
                        in_=ps[:rows, :128],
                    )

            # ---- phase B: gather+transpose Xq -> xqT [128, 3, TQ] ----
            xqT = consts.tile([128, 3, TQ], F32R, tag="xqT")
            for t in range(TQ_TILES):
                gather_transpose(idxq_sb, t, xqT, t)

            # ---- phase C: hqT [128, DT, TQ] (fp32; only DVE consumes it) ----
            hqT = consts.tile([128, DT, TQ], F32, tag="hqT")
            for m in range(DT):
                ps = mm_p.tile([128, 512], F32, tag="mm")
                for c, rows in enumerate(E_CHUNKS):
                    nc.tensor.matmul(
                        ps[:, :TQ],
                        lhsT=wq_sb[:rows, c, m * 128 : (m + 1) * 128],
                        rhs=xqT[:rows, c, :],
                        start=(c == 0),
                        stop=(c == 2),
                    )
                nc.vector.tensor_copy(hqT[:, m, :], ps[:, :TQ])

            poT = consts.tile([128, DT, G, BL], F32R, tag="poT")

            # ---- phase D: attention, two samples per D2 batch ----
            for p in range(BL // 2):
                # D1: gather + transpose K tokens for samples 2p, 2p+1
                xkT = xkt_p.tile([128, 3, 512], F32R, tag="xkT")
                for t in range(4):
                    gather_transpose(idxk_sb, p * 4 + t, xkT, t)

                # D2: hkT for the pair [128, DT, 512]
                hkT = hkt_p.tile([128, DT, 512], F32R, tag="hkT")
                for m in range(DT):
                    ps = mm_p.tile([128, 512], F32, tag="mm")
                    for c, rows in enumerate(E_CHUNKS):
                        nc.tensor.matmul(
                            ps[:],
                            lhsT=wk_sb[:rows, c, m * 128 : (m + 1) * 128],
                            rhs=xkT[:rows, c, :],
                            start=(c == 0),
                            stop=(c == 2),
                        )
                    nc.any.tensor_copy(out=hkT[:, m, :], in_=ps[:])

                for bi in range(2):
                    b = p * 2 + bi

                    # D3: hk_b [128, 2, H] (token-partition layout)
                    hk = hk_p.tile([128, 2, H], F32R, tag="hk")
                    for t in range(2):
                        for nchunk in range(2):
                            ps = mm_p.tile([128, 512], F32, tag="mm")
                            for c, rows in enumerate(E_CHUNKS):
                                nc.tensor.matmul(
                                    ps[:],
                                    lhsT=xkT[
                                        :rows, c,
                                        (bi * 2 + t) * 128 : (bi * 2 + t + 1) * 128,
                                    ],
                                    rhs=wk_sb[
                                        :rows, c, nchunk * 512 : (nchunk + 1) * 512
                                    ],
                                    start=(c == 0),
                                    stop=(c == 2),
                                )
                            nc.any.tensor_copy(
                                out=hk[:, t, nchunk * 512 : (nchunk + 1) * 512],
                                in_=ps[:],
                            )

                    # D4: hqw [128, DT, G, NQ] = hqT(b) * watt (bcast over g)
                    hqw = hqw_p.tile([128, DT, G, NQ], F32R, tag="hqw")
                    nc.gpsimd.tensor_tensor(
                        out=hqw[:],
                        in0=hqT[:, :, None, b * NQ : (b + 1) * NQ].to_broadcast(
                            [128, DT, G, NQ]
                        ),
                        in1=watt_sb[:, :, :, None].to_broadcast([128, DT, G, NQ]),
                        op=OP.mult,
                    )

                    # D5: logits.T [k, (g,q)] in PSUM: [128, 2, 256]
                    ps_l = lg_p.tile([128, 512], F32, tag="lg")
                    for kt in range(2):
                        for c in range(DT):
                            nc.tensor.matmul(
                                ps_l[:, kt * 256 : (kt + 1) * 256],
                                lhsT=hkT[
                                    :, c,
                                    bi * 256 + kt * 128 : bi * 256 + (kt + 1) * 128,
                                ],
                                rhs=hqw[:, c],
                                start=(c == 0),
                                stop=(c == DT - 1),
                            )

                    # D6: E = exp(logits), per-(g)-block row sums zz
                    et = et_p.tile([128, 2, G * NQ], F32R, tag="et")
                    zz = zz_p.tile([128, 2, G], F32R, tag="zz")
                    for kt in range(2):
                        nc.scalar.activation(
                            out=et[:, kt, :],
                            in_=ps_l[:, kt * 256 : (kt + 1) * 256],
                            func=AF.Exp,
                        )
                        with nc.allow_low_precision(reason="fp32r round of f32 sum"):
                            nc.vector.tensor_reduce(
                                out=zz[:, kt, :],
                                in_=et[:, kt].rearrange("p (g q) -> p g q", g=G),
                                axis=AX.X,
                                op=OP.add,
                            )

                    # D7: Z_g = sum over k-partitions; zinv_b = 1/Z broadcast
                    ps_z = mm_p.tile([128, 512], F32, tag="mm")
                    for kt in range(2):
                        nc.tensor.matmul(
                            ps_z[:1, :G],
                            lhsT=ones_sb[:],
                            rhs=zz[:, kt, :],
                            start=(kt == 0),
                            stop=(kt == 1),
                        )
                    zinv = zn_p.tile([1, G], F32, tag="zinv")
                    nc.vector.reciprocal(zinv[:1, :], ps_z[:1, :G])
                    zbro = zn_p.tile([128, G], F32, tag="zbro")
                    nc.gpsimd.partition_broadcast(zbro[:], zinv[:1, :], channels=128)

                    # D8: u.T, v, pooled partial sums; 2 d-tiles per PSUM tile
                    for mp in range(4):
                        ps_u = up_p.tile([128, 512], F32, tag="up")
                        for mi in range(2):
                            m = mp * 2 + mi
                            for kt in range(2):
                                nc.tensor.matmul(
                                    ps_u[:, mi * 256 : (mi + 1) * 256],
                                    lhsT=hk[:, kt, m * 128 : (m + 1) * 128],
                                    rhs=et[:, kt, :],
                                    start=(kt == 0),
                                    stop=(kt == 1),
                                )
                        v = v_p.tile([128, 2, G, NQ], F32, tag="v")
                        nc.vector.tensor_tensor(
                            out=v[:],
                            in0=ps_u[:].rearrange("p (m g q) -> p m g q", m=2, g=G),
                            in1=hqT[
                                :, mp * 2 : mp * 2 + 2, None, b * NQ : (b + 1) * NQ
                            ].to_broadcast([128, 2, G, NQ]),
                            op=OP.mult,
                        )
                        vr = v_p.tile([128, 2, G], F32, tag="vr")
                        nc.vector.tensor_reduce(
                            out=vr[:], in_=v[:], axis=AX.X, op=OP.add
                        )
                        with nc.allow_low_precision(reason="fp32r round"):
                            nc.vector.tensor_tensor(
                                out=poT[:, mp * 2 : mp * 2 + 2, :, b],
                                in0=vr[:],
                                in1=zbro[:, None, :].to_broadcast([128, 2, G]),
                                op=OP.mult,
                            )

            # ---- phase F: out [8, 300] = pooled_flat @ Wout + bout ----
            ps_o = mm_p.tile([128, 512], F32, tag="mm")
            for g in range(G):
                for m in range(DT):
                    t = g * DT + m
                    nc.tensor.matmul(
                        ps_o[:BL, :N_OUT],
                        lhsT=poT[:, m, g, :],
                        rhs=wout_tiles[t][:],
                        start=(t == 0),
                        stop=(t == NWOUT - 1),
                    )
            out_sb = consts.tile([BL, N_OUT], F32R, tag="out_sb")
            nc.vector.tensor_tensor(
                out=out_sb[:], in0=ps_o[:BL, :N_OUT], in1=bout_sb[:], op=OP.add
            )

            # ---- phase G: sim + log_softmax ----
            outT = consts.tile([128, 3, BL], F32R, tag="outT")
            for c, rows in enumerate(N_CHUNKS):
                ps = mm_p.tile([128, 512], F32R, tag="mm")
                nc.tensor.transpose(
                    ps[:rows, :BL],
                    out_sb[:, c * 128 : c * 128 + rows],
                    ident[:BL, :BL],
                )
                nc.vector.tensor_copy(outT[:rows, c, :], ps[:rows, :BL])

            sim_sb = consts.tile([BL, N_ANS], F32, tag="sim_sb")
            esc = consts.tile([BL, NA_W], F32, tag="esc")
            mx8 = consts.tile([BL, NA_CH], F32, tag="mx8")
            zs8 = consts.tile([BL, NA_CH], F32, tag="zs8")
            mx = consts.tile([BL, 1], F32, tag="mx")
            nmx = consts.tile([BL, 1], F32, tag="nmx")
            zs = consts.tile([BL, 1], F32, tag="zs")
            lnz = consts.tile([BL, 1], F32, tag="lnz")
            for a in range(NA_CH):
                ps_s = mm_p.tile([128, 512], F32, tag="mm")
                for c, rows in enumerate(N_CHUNKS):
                    nc.tensor.matmul(
                        ps_s[:BL, :NA_W],
                        lhsT=outT[:rows, c, :],
                        rhs=glove_tiles[a][:rows, c, :],
                        start=(c == 0),
                        stop=(c == 2),
                    )
                nc.vector.tensor_reduce(
                    out=mx8[:, a : a + 1], in_=ps_s[:BL, :NA_W], axis=AX.X, op=OP.max
                )
                nc.vector.tensor_copy(sim_sb[:, a * NA_W : (a + 1) * NA_W],
                                      ps_s[:BL, :NA_W])
            nc.vector.tensor_reduce(out=mx[:], in_=mx8[:], axis=AX.X, op=OP.max)
            nc.vector.tensor_scalar_mul(nmx[:], mx[:], -1.0)
            for a in range(NA_CH):
                nc.scalar.activation(
                    out=esc[:],  # scratch, discarded
                    in_=sim_sb[:, a * NA_W : (a + 1) * NA_W],
                    func=AF.Exp,
                    bias=nmx[:],
                    accum_out=zs8[:, a : a + 1],
                )
            nc.vector.tensor_reduce(out=zs[:], in_=zs8[:], axis=AX.X, op=OP.add)
            nc.scalar.activation(out=lnz[:], in_=zs[:], func=AF.Ln)
            nc.vector.tensor_scalar(
                out=sim_sb[:],
                in0=sim_sb[:],
                scalar1=mx[:],
                scalar2=lnz[:],
                op0=OP.subtract,
                op1=OP.subtract,
            )
            nc.sync.dma_start(out_d, sim_sb[:])

    nc.compile()
    return nc


_NC = None


def _get_nc():
    global _NC
    if _NC is None:
        _NC = build_kernel()
    return _NC


def make_in_maps(inputs):
    he_q = np.asarray(inputs["he_ques"]).astype(np.int32)   # [64, 32]
    he_k = np.asarray(inputs["he_kg"]).astype(np.int32)     # [64, 256]
    emb0 = np.asarray(inputs["emb"], dtype=np.float32)
    emb = np.ones((VOCAB, EA), dtype=np.float32)            # ones col at E=300
    emb[:, :E] = emb0
    wq = np.concatenate(
        [np.asarray(inputs["Wq"], np.float32),
         np.asarray(inputs["bq"], np.float32)[None, :]], axis=0)
    wk = np.concatenate(
        [np.asarray(inputs["Wk"], np.float32),
         np.asarray(inputs["bk"], np.float32)[None, :]], axis=0)
    watt = np.ascontiguousarray(
        np.asarray(inputs["Watt"], np.float32).reshape(DT, 128, G)
        .transpose(1, 0, 2))                                # [128, DT, G]
    wout = np.ascontiguousarray(np.asarray(inputs["Wout"], np.float32))
    bout = np.ascontiguousarray(
        np.broadcast_to(np.asarray(inputs["bout"], np.float32), (BL, N_OUT)))
    glovet = np.ascontiguousarray(
        np.asarray(inputs["glove_cands"], np.float32).T)    # [300, 4000]
    ident = np.eye(128, dtype=np.float32)

    in_maps = []
    for i in range(N_CORES):
        iq = he_q[i * BL : (i + 1) * BL].reshape(-1)        # [256]
        ik = he_k[i * BL : (i + 1) * BL].reshape(-1)        # [2048]
        in_maps.append({
            "emb": emb,
            "idx_q": np.ascontiguousarray(iq.reshape(TQ_TILES, 128).T),
            "idx_k": np.ascontiguousarray(ik.reshape(TK_TILES, 128).T),
            "wq": wq,
            "wk": wk,
            "watt": watt,
            "wout": wout,
            "bout": bout,
            "glovet": glovet,
            "ident": ident,
            "ones_col": np.ones((128, 1), dtype=np.float32),
        })
    return in_maps


def kernel(**inputs) -> np.ndarray:
    nc = _get_nc()
    in_maps = make_in_maps(inputs)
    res = run_bass_kernel_spmd(nc, in_maps, list(range(N_CORES)))
    return np.concatenate([res.results[i]["out"] for i in range(N_CORES)], axis=0)


# revision 16
# speedup vs baseline: 1.3474x; 1.3474x over previous
"""Trainium2 Bass kernel for the BAN (bilinear attention network) problem.

Math (per batch b, eval mode):
    hq = emb[he_ques] @ Wq + bq                  [NQ, H]
    hk = emb[he_kg]   @ Wk + bk                  [NK, H]
    logits[g,q,k] = sum_d hq[q,d] Watt[d,g] hk[k,d]   (+ batt[g], which cancels
                                                       in the joint softmax)
    att = softmax over flattened (q,k) per (b,g)
    pooled[g,d] = sum_{q,k} hq[q,d] att[g,q,k] hk[k,d]
    out = pooled.flat @ Wout + bout;  sim = out @ glove.T;  log_softmax(sim)

Distribution: pure data parallel over batch, 8 samples per core on 8 cores.
All weights replicated. No collectives.

Matmul operands use float32r (single-pass PE mode, ~1.3e-4 matmul accuracy
vs 4x slower two-pass fp32); accumulation stays fp32 in PSUM.

Layout strategy (per core, B_loc=8):
  - emb is host-augmented with a ones column (E=301) so projection biases
    ride in the matmul; tokens gathered on-device via indirect DMA:
    X [token, 301], PE-transposed to X.T [E, token].
  - hq.T [d, tok] and hk.T [d, tok] from lhsT=W, rhs=X.T
  - hk   [tok, d] from lhsT=X.T, rhs=W
  - logits.T [k, (g,q)] = (hk.T tiles).T @ (hq.T * Watt[:,g])  -> logits are
    O(+-6), so exp() without max subtraction is safe in fp32; the joint
    softmax normalization Z_g = sum E is applied to pooled.T per sample.
  - u.T [d, (g,q)] = (hk tiles).T @ E.T;  v = u.T * hq.T (bcast over g);
    pooled.T[d, g] = reduce_q v, scaled by 1/Z_g.
  - out [8, 300] = (pooled.T as lhsT).T @ Wout tiles;  sim [8, 4000] via
    lhsT=out.T (PE transpose), rhs=glove.T;  log-softmax on [8, 4000].
  - Wout/glove.T tile DMAs are emitted up-front so they stream into SBUF
    while the attention loop runs.
"""

import sys

if "/opt/trn_rl_repo" not in sys.path:
    sys.path.insert(0, "/opt/trn_rl_repo")

import numpy as np

import concourse.bass as bass
import concourse.mybir as mybir
import concourse.tile as tile
from concourse import bacc
from concourse.bass_utils import run_bass_kernel_spmd

F32 = mybir.dt.float32
F32R = mybir.dt.float32r
I32 = mybir.dt.int32
AX = mybir.AxisListType
OP = mybir.AluOpType
AF = mybir.ActivationFunctionType

N_CORES = 8
VOCAB = 20000
E = 300          # word embedding size
EA = E + 1       # augmented with ones column (bias trick)
H = 1024         # hidden
G = 8            # heads
N_OUT = 300
N_ANS = 4000
B, NQ, NK = 64, 32, 256
BL = B // N_CORES            # 8 samples per core
TQ = BL * NQ                 # 256 q tokens per core
TK = BL * NK                 # 2048 k tokens per core
TQ_TILES = TQ // 128         # 2
TK_TILES = TK // 128         # 16
DT = H // 128                # 8 d-tiles
E_CHUNKS = (128, 128, EA - 256)   # (128, 128, 45)
N_CHUNKS = (128, 128, N_OUT - 256)
NA_CH = 8                    # sim computed in 8 chunks of 500
NA_W = N_ANS // NA_CH        # 500
NWOUT = G * DT               # 64 Wout k-tiles


def build_kernel():
    nc = bacc.Bacc("TRN2", target_bir_lowering=False, debug=False,
                   num_devices=N_CORES)

    # ---- DRAM I/O ----
    emb_d = nc.dram_tensor("emb", [VOCAB, EA], F32R, kind="ExternalInput").ap()
    idxq_d = nc.dram_tensor("idx_q", [128, TQ_TILES], I32, kind="ExternalInput").ap()
    idxk_d = nc.dram_tensor("idx_k", [128, TK_TILES], I32, kind="ExternalInput").ap()
    wq_d = nc.dram_tensor("wq", [EA, H], F32R, kind="ExternalInput").ap()
    wk_d = nc.dram_tensor("wk", [EA, H], F32R, kind="ExternalInput").ap()
    watt_d = nc.dram_tensor("watt", [128, DT, G], F32, kind="ExternalInput").ap()
    wout_d = nc.dram_tensor("wout", [G * H, N_OUT], F32R, kind="ExternalInput").ap()
    bout_d = nc.dram_tensor("bout", [BL, N_OUT], F32, kind="ExternalInput").ap()
    glovet_d = nc.dram_tensor("glovet", [N_OUT, N_ANS], F32R,
                              kind="ExternalInput").ap()
    ident_d = nc.dram_tensor("ident", [128, 128], F32R, kind="ExternalInput").ap()
    ones_d = nc.dram_tensor("ones_col", [128, 1], F32R, kind="ExternalInput").ap()
    out_d = nc.dram_tensor("out", [BL, N_ANS], F32, kind="ExternalOutput").ap()

    with tile.TileContext(nc) as tc:
        import contextlib

        with contextlib.ExitStack() as ctx:
            consts = ctx.enter_context(tc.tile_pool(name="consts", bufs=1))
            xrow_p = ctx.enter_context(tc.tile_pool(name="xrow", bufs=4))
            xkt_p = ctx.enter_context(tc.tile_pool(name="xkt", bufs=2))
            hkt_p = ctx.enter_context(tc.tile_pool(name="hkt", bufs=2))
            hk_p = ctx.enter_context(tc.tile_pool(name="hk", bufs=2))
            hqw_p = ctx.enter_context(tc.tile_pool(name="hqw", bufs=2))
            et_p = ctx.enter_context(tc.tile_pool(name="et", bufs=2))
            v_p = ctx.enter_context(tc.tile_pool(name="v", bufs=2))
            zz_p = ctx.enter_context(tc.tile_pool(name="zz", bufs=2))
            zn_p = ctx.enter_context(tc.tile_pool(name="zn", bufs=2))
            wout_p = ctx.enter_context(tc.tile_pool(name="wout", bufs=24))
            glove_p = ctx.enter_context(tc.tile_pool(name="glove", bufs=4))
            mm_p = ctx.enter_context(tc.tile_pool(name="mm", bufs=4, space="PSUM"))
            lg_p = ctx.enter_context(tc.tile_pool(name="lg", bufs=2, space="PSUM"))
            up_p = ctx.enter_context(tc.tile_pool(name="up", bufs=2, space="PSUM"))

            # ---- constants into SBUF ----
            ident = consts.tile([128, 128], F32R, tag="ident")
            nc.sync.dma_start(ident[:], ident_d)
            idxq_sb = consts.tile([128, TQ_TILES], I32, tag="idxq")
            nc.sync.dma_start(idxq_sb[:], idxq_d)
            idxk_sb = consts.tile([128, TK_TILES], I32, tag="idxk")
            nc.sync.dma_start(idxk_sb[:], idxk_d)
            wq_sb = consts.tile([128, 3, H], F32R, tag="wq")
            wk_sb = consts.tile([128, 3, H], F32R, tag="wk")
            for c, rows in enumerate(E_CHUNKS):
                nc.sync.dma_start(wq_sb[:rows, c, :], wq_d[c * 128 : c * 128 + rows])
                nc.sync.dma_start(wk_sb[:rows, c, :], wk_d[c * 128 : c * 128 + rows])
            watt_sb = consts.tile([128, DT, G], F32, tag="watt")
            nc.sync.dma_start(watt_sb[:], watt_d)
            bout_sb = consts.tile([BL, N_OUT], F32, tag="bout")
            nc.sync.dma_start(bout_sb[:], bout_d)
            ones_sb = consts.tile([128, 1], F32R, tag="ones")
            nc.sync.dma_start(ones_sb[:], ones_d)

            # ---- early-issued weight streams for phases F/G ----
            wout_tiles = []
            for t in range(NWOUT):
                wtile = wout_p.tile([128, N_OUT], F32R, tag="wout")
                nc.sync.dma_start(wtile[:], wout_d[t * 128 : (t + 1) * 128, :])
                wout_tiles.append(wtile)
            glove_tiles = []
            for a in range(NA_CH):
                gt = glove_p.tile([128, 3, NA_W], F32R, tag="glove")
                for c, rows in enumerate(N_CHUNKS):
                    nc.sync.dma_start(
                        gt[:rows, c, :],
                        glovet_d[c * 128 : c * 128 + rows,
                                 a * NA_W : (a + 1) * NA_W],
                    )
                glove_tiles.append(gt)

            def gather_transpose(idx_sb, col, dst, dst_col):
                """Gather 128 emb rows (token tile) and write transpose into
                dst[:, c, dst_col*128:...] per E-chunk c. The emb ones column
                (E index 300) lands at partition 44 of chunk 2."""
                xrow = xrow_p.tile([128, EA], F32R, tag="xrow")
                nc.gpsimd.indirect_dma_start(
                    out=xrow[:],
                    out_offset=None,
                    in_=emb_d,
                    in_offset=bass.IndirectOffsetOnAxis(
                        ap=idx_sb[:, col : col + 1], axis=0
                    ),
                )
                for c, rows in enumerate(E_CHUNKS):
                    ps = mm_p.tile([128, 512], F32R, tag="mm")
                    nc.tensor.transpose(
                        ps[:rows, :128], xrow[:, c * 128 : c * 128 + rows], ident[:]
                    )
                    nc.any.tensor_copy(
                        out=dst[:rows, c, dst_col * 128 : (dst_col + 1) * 128],
                        in_=ps[:rows, :128],
                    )

            # ---- phase B: gather+transpose Xq -> xqT [128, 3, TQ] ----
            xqT = consts.tile([128, 3, TQ], F32R, tag="xqT")
            for t in range(TQ_TILES):
                gather_transpose(idxq_sb, t, xqT, t)

            # ---- phase C: hqT [128, DT, TQ] (fp32; only DVE consumes it) ----
            hqT = consts.tile([128, DT, TQ], F32, tag="hqT")
            for m in range(DT):
                ps = mm_p.tile([128, 512], F32, tag="mm")
                for c, rows in enumerate(E_CHUNKS):
                    nc.tensor.matmul(
                        ps[:, :TQ],
                        lhsT=wq_sb[:rows, c, m * 128 : (m + 1) * 128],
                        rhs=xqT[:rows, c, :],
                        start=(c == 0),
                        stop=(c == 2),
                    )
                nc.vector.tensor_copy(hqT[:, m, :], ps[:, :TQ])

            poT = consts.tile([128, DT, G, BL], F32R, tag="poT")

            # ---- phase D: attention, two samples per D2 batch ----
            for p in range(BL // 2):
                # D1: gather + transpose K tokens for samples 2p, 2p+1
                xkT = xkt_p.tile([128, 3, 512], F32R, tag="xkT")
                for t in range(4):
                    gather_transpose(idxk_sb, p * 4 + t, xkT, t)

                # D2: hkT for the pair [128, DT, 512]
                hkT = hkt_p.tile([128, DT, 512], F32R, tag="hkT")
                for m in range(DT):
                    ps = mm_p.tile([128, 512], F32, tag="mm")
                    for c, rows in enumerate(E_CHUNKS):
                        nc.tensor.matmul(
                            ps[:],
                            lhsT=wk_sb[:rows, c, m * 128 : (m + 1) * 128],
                            rhs=xkT[:rows, c, :],
                            start=(c == 0),
                            stop=(c == 2),
                        )
                    nc.any.tensor_copy(out=hkT[:, m, :], in_=ps[:])

                for bi in range(2):
                    b = p * 2 + bi

                    # D3: hk_b [128, 2, H] (token-partition layout)
                    hk = hk_p.tile([128, 2, H], F32R, tag="hk")
                    for t in range(2):
                        for nchunk in range(2):
                            ps = mm_p.tile([128, 512], F32, tag="mm")
                            for c, rows in enumerate(E_CHUNKS):
                                nc.tensor.matmul(
                                    ps[:],
                                    lhsT=xkT[
                                        :rows, c,
                                        (bi * 2 + t) * 128 : (bi * 2 + t + 1) * 128,
                                    ],
                                    rhs=wk_sb[
                                        :rows, c, nchunk * 512 : (nchunk + 1) * 512
                                    ],
                                    start=(c == 0),
                                    stop=(c == 2),
                                )
                            nc.any.tensor_copy(
                                out=hk[:, t, nchunk * 512 : (nchunk + 1) * 512],
                                in_=ps[:],
                            )

                    # D4: hqw [128, DT, G, NQ] = hqT(b) * watt (bcast over g)
                    hqw = hqw_p.tile([128, DT, G, NQ], F32R, tag="hqw")
                    nc.vector.tensor_tensor(
                        out=hqw[:],
                        in0=hqT[:, :, None, b * NQ : (b + 1) * NQ].to_broadcast(
                            [128, DT, G, NQ]
                        ),
                        in1=watt_sb[:, :, :, None].to_broadcast([128, DT, G, NQ]),
                        op=OP.mult,
                    )

                    # D5: logits.T [k, (g,q)] in PSUM: [128, 2, 256]
                    ps_l = lg_p.tile([128, 512], F32, tag="lg")
                    for kt in range(2):
                        for c in range(DT):
                            nc.tensor.matmul(
                                ps_l[:, kt * 256 : (kt + 1) * 256],
                                lhsT=hkT[
                                    :, c,
                                    bi * 256 + kt * 128 : bi * 256 + (kt + 1) * 128,
                                ],
                                rhs=hqw[:, c],
                                start=(c == 0),
                                stop=(c == DT - 1),
                            )

                    # D6: E = exp(logits), per-(g)-block row sums zz
                    et = et_p.tile([128, 2, G * NQ], F32R, tag="et")
                    zz = zz_p.tile([128, 2, G], F32R, tag="zz")
                    for kt in range(2):
                        nc.scalar.activation(
                            out=et[:, kt, :],
                            in_=ps_l[:, kt * 256 : (kt + 1) * 256],
                            func=AF.Exp,
                        )
                        with nc.allow_low_precision(reason="fp32r round of f32 sum"):
                            nc.vector.tensor_reduce(
                                out=zz[:, kt, :],
                                in_=et[:, kt].rearrange("p (g q) -> p g q", g=G),
                                axis=AX.X,
                                op=OP.add,
                            )

                    # D7: Z_g = sum over k-partitions; zinv_b = 1/Z broadcast
                    ps_z = mm_p.tile([128, 512], F32, tag="mm")
                    for kt in range(2):
                        nc.tensor.matmul(
                            ps_z[:1, :G],
                            lhsT=ones_sb[:],
                            rhs=zz[:, kt, :],
                            start=(kt == 0),
                            stop=(kt == 1),
                        )
                    zinv = zn_p.tile([1, G], F32, tag="zinv")
                    nc.vector.reciprocal(zinv[:1, :], ps_z[:1, :G])
                    zbro = zn_p.tile([128, G], F32, tag="zbro")
                    nc.gpsimd.partition_broadcast(zbro[:], zinv[:1, :], channels=128)

                    # D8: u.T, v, pooled partial sums; 2 d-tiles per PSUM tile
                    for mp in range(4):
                        ps_u = up_p.tile([128, 512], F32, tag="up")
                        for mi in range(2):
                            m = mp * 2 + mi
                            for kt in range(2):
                                nc.tensor.matmul(
                                    ps_u[:, mi * 256 : (mi + 1) * 256],
                                    lhsT=hk[:, kt, m * 128 : (m + 1) * 128],
                                    rhs=et[:, kt, :],
                                    start=(kt == 0),
                                    stop=(kt == 1),
                                )
                        v = v_p.tile([128, 2, G, NQ], F32, tag="v")
                        nc.vector.tensor_tensor(
                            out=v[:],
                            in0=ps_u[:].rearrange("p (m g q) -> p m g q", m=2, g=G),
                            in1=hqT[
                                :, mp * 2 : mp * 2 + 2, None, b * NQ : (b + 1) * NQ
                            ].to_broadcast([128, 2, G, NQ]),
                            op=OP.mult,
                        )
                        vr = v_p.tile([128, 2, G], F32, tag="vr")
                        nc.vector.tensor_reduce(
                            out=vr[:], in_=v[:], axis=AX.X, op=OP.add
                        )
                        with nc.allow_low_precision(reason="fp32r round"):
                            nc.vector.tensor_tensor(
                                out=poT[:, mp * 2 : mp * 2 + 2, :, b],
                                in0=vr[:],
                                in1=zbro[:, None, :].to_broadcast([128, 2, G]),
                                op=OP.mult,
                            )

            # ---- phase F: out [8, 300] = pooled_flat @ Wout + bout ----
            ps_o = mm_p.tile([128, 512], F32, tag="mm")
            for g in range(G):
                for m in range(DT):
                    t = g * DT + m
                    nc.tensor.matmul(
                        ps_o[:BL, :N_OUT],
                        lhsT=poT[:, m, g, :],
                        rhs=wout_tiles[t][:],
                        start=(t == 0),
                        stop=(t == NWOUT - 1),
                    )
            out_sb = consts.tile([BL, N_OUT], F32R, tag="out_sb")
            nc.vector.tensor_tensor(
                out=out_sb[:], in0=ps_o[:BL, :N_OUT], in1=bout_sb[:], op=OP.add
            )

            # ---- phase G: sim + log_softmax ----
            outT = consts.tile([128, 3, BL], F32R, tag="outT")
            for c, rows in enumerate(N_CHUNKS):
                ps = mm_p.tile([128, 512], F32R, tag="mm")
                nc.tensor.transpose(
                    ps[:rows, :BL],
                    out_sb[:, c * 128 : c * 128 + rows],
                    ident[:BL, :BL],
                )
                nc.vector.tensor_copy(outT[:rows, c, :], ps[:rows, :BL])

            sim_sb = consts.tile([BL, N_ANS], F32, tag="sim_sb")
            esc = consts.tile([BL, NA_W], F32, tag="esc")
            mx8 = consts.tile([BL, NA_CH], F32, tag="mx8")
            zs8 = consts.tile([BL, NA_CH], F32, tag="zs8")
            mx = consts.tile([BL, 1], F32, tag="mx")
            nmx = consts.tile([BL, 1], F32, tag="nmx")
            zs = consts.tile([BL, 1], F32, tag="zs")
            lnz = consts.tile([BL, 1], F32, tag="lnz")
            for a in range(NA_CH):
                ps_s = mm_p.tile([128, 512], F32, tag="mm")
                for c, rows in enumerate(N_CHUNKS):
                    nc.tensor.matmul(
                        ps_s[:BL, :NA_W],
                        lhsT=outT[:rows, c, :],
                        rhs=glove_tiles[a][:rows, c, :],
                        start=(c == 0),
                        stop=(c == 2),
                    )
                nc.vector.tensor_reduce(
                    out=mx8[:, a : a + 1], in_=ps_s[:BL, :NA_W], axis=AX.X, op=OP.max
                )
                nc.vector.tensor_copy(sim_sb[:, a * NA_W : (a + 1) * NA_W],
                                      ps_s[:BL, :NA_W])
            nc.vector.tensor_reduce(out=mx[:], in_=mx8[:], axis=AX.X, op=OP.max)
            nc.vector.tensor_scalar_mul(nmx[:], mx[:], -1.0)
            for a in range(NA_CH):
                nc.scalar.activation(
                    out=esc[:],  # scratch, discarded
                    in_=sim_sb[:, a * NA_W : (a + 1) * NA_W],
                    func=AF.Exp,
                    bias=nmx[:],
                    accum_out=zs8[:, a : a + 1],
                )
            nc.vector.tensor_reduce(out=zs[:], in_=zs8[:], axis=AX.X, op=OP.add)
            nc.scalar.activation(out=lnz[:], in_=zs[:], func=AF.Ln)
            nc.vector.tensor_scalar(
                out=sim_sb[:],
                in0=sim_sb[:],
                scalar1=mx[:],
                scalar2=lnz[:],
                op0=OP.subtract,
                op1=OP.subtract,
            )
            nc.sync.dma_start(out_d, sim_sb[:])

    nc.compile()
    return nc


_NC = None


def _get_nc():
    global _NC
    if _NC is None:
        _NC = build_kernel()
    return _NC


def make_in_maps(inputs):
    he_q = np.asarray(inputs["he_ques"]).astype(np.int32)   # [64, 32]
    he_k = np.asarray(inputs["he_kg"]).astype(np.int32)     # [64, 256]
    emb0 = np.asarray(inputs["emb"], dtype=np.float32)
    emb = np.ones((VOCAB, EA), dtype=np.float32)            # ones col at E=300
    emb[:, :E] = emb0
    wq = np.concatenate(
        [np.asarray(inputs["Wq"], np.float32),
         np.asarray(inputs["bq"], np.float32)[None, :]], axis=0)
    wk = np.concatenate(
        [np.asarray(inputs["Wk"], np.float32),
         np.asarray(inputs["bk"], np.float32)[None, :]], axis=0)
    watt = np.ascontiguousarray(
        np.asarray(inputs["Watt"], np.float32).reshape(DT, 128, G)
        .transpose(1, 0, 2))                                # [128, DT, G]
    wout = np.ascontiguousarray(np.asarray(inputs["Wout"], np.float32))
    bout = np.ascontiguousarray(
        np.broadcast_to(np.asarray(inputs["bout"], np.float32), (BL, N_OUT)))
    glovet = np.ascontiguousarray(
        np.asarray(inputs["glove_cands"], np.float32).T)    # [300, 4000]
    ident = np.eye(128, dtype=np.float32)

    in_maps = []
    for i in range(N_CORES):
        iq = he_q[i * BL : (i + 1) * BL].reshape(-1)        # [256]
        ik = he_k[i * BL : (i + 1) * BL].reshape(-1)        # [2048]
        in_maps.append({
            "emb": emb,
            "idx_q": np.ascontiguousarray(iq.reshape(TQ_TILES, 128).T),
            "idx_k": np.ascontiguousarray(ik.reshape(TK_TILES, 128).T),
            "wq": wq,
            "wk": wk,
            "watt": watt,
            "wout": wout,
            "bout": bout,
            "glovet": glovet,
            "ident": ident,
            "ones_col": np.ones((128, 1), dtype=np.float32),
        })
    return in_maps


def kernel(**inputs) -> np.ndarray:
    nc = _get_nc()
    in_maps = make_in_maps(inputs)
    res = run_bass_kernel_spmd(nc, in_maps, list(range(N_CORES)))
    return np.concatenate([res.results[i]["out"] for i in range(N_CORES)], axis=0)


# revision 17
# speedup vs baseline: 1.3588x; 1.0084x over previous
"""Trainium2 Bass kernel for the BAN (bilinear attention network) problem.

Math (per batch b, eval mode):
    hq = emb[he_ques] @ Wq + bq                  [NQ, H]
    hk = emb[he_kg]   @ Wk + bk                  [NK, H]
    logits[g,q,k] = sum_d hq[q,d] Watt[d,g] hk[k,d]   (+ batt[g], which cancels
                                                       in the joint softmax)
    att = softmax over flattened (q,k) per (b,g)
    pooled[g,d] = sum_{q,k} hq[q,d] att[g,q,k] hk[k,d]
    out = pooled.flat @ Wout + bout;  sim = out @ glove.T;  log_softmax(sim)

Distribution: pure data parallel over batch, 8 samples per core on 8 cores.
All weights replicated. No collectives.

Matmul operands use float32r (single-pass PE mode, ~1.3e-4 matmul accuracy
vs 4x slower two-pass fp32); accumulation stays fp32 in PSUM.

Layout strategy (per core, B_loc=8):
  - emb is host-augmented with a ones column (E=301) so projection biases
    ride in the matmul; tokens gathered on-device via indirect DMA:
    X [token, 301], PE-transposed to X.T [E, token].
  - hq.T [d, tok] and hk.T [d, tok] from lhsT=W, rhs=X.T
  - hk   [tok, d] from lhsT=X.T, rhs=W
  - logits.T [k, (g,q)] = (hk.T tiles).T @ (hq.T * Watt[:,g])  -> logits are
    O(+-6), so exp() without max subtraction is safe in fp32; the joint
    softmax normalization Z_g = sum E is applied to pooled.T per sample.
  - u.T [d, (g,q)] = (hk tiles).T @ E.T;  v = u.T * hq.T (bcast over g);
    pooled.T[d, g] = reduce_q v, scaled by 1/Z_g.
  - out [8, 300] = (pooled.T as lhsT).T @ Wout tiles;  sim [8, 4000] via
    lhsT=out.T (PE transpose), rhs=glove.T;  log-softmax on [8, 4000].
  - Wout/glove.T tile DMAs are emitted up-front so they stream into SBUF
    while the attention loop runs.
"""

import sys

if "/opt/trn_rl_repo" not in sys.path:
    sys.path.insert(0, "/opt/trn_rl_repo")

import numpy as np

import concourse.bass as bass
import concourse.mybir as mybir
import concourse.tile as tile
from concourse import bacc
from concourse.bass_utils import run_bass_kernel_spmd

F32 = mybir.dt.float32
F32R = mybir.dt.float32r
I32 = mybir.dt.int32
AX = mybir.AxisListType
OP = mybir.AluOpType
AF = mybir.ActivationFunctionType

N_CORES = 8
VOCAB = 20000
E = 300          # word embedding size
EA = E + 1       # augmented with ones column (bias trick)
H = 1024         # hidden
G = 8            # heads
N_OUT = 300
N_ANS = 4000
B, NQ, NK = 64, 32, 256
BL = B // N_CORES            # 8 samples per core
TQ = BL * NQ                 # 256 q tokens per core
TK = BL * NK                 # 2048 k tokens per core
TQ_TILES = TQ // 128         # 2
TK_TILES = TK // 128         # 16
DT = H // 128                # 8 d-tiles
E_CHUNKS = (128, 128, EA - 256)   # (128, 128, 45)
N_CHUNKS = (128, 128, N_OUT - 256)
NA_CH = 8                    # sim computed in 8 chunks of 500
NA_W = N_ANS // NA_CH        # 500
NWOUT = G * DT               # 64 Wout k-tiles


def build_kernel():
    nc = bacc.Bacc("TRN2", target_bir_lowering=False, debug=False,
                   num_devices=N_CORES)

    # ---- DRAM I/O ----
    emb_d = nc.dram_tensor("emb", [VOCAB, EA], F32R, kind="ExternalInput").ap()
    idxq_d = nc.dram_tensor("idx_q", [128, TQ_TILES], I32, kind="ExternalInput").ap()
    idxk_d = nc.dram_tensor("idx_k", [128, TK_TILES], I32, kind="ExternalInput").ap()
    wq_d = nc.dram_tensor("wq", [EA, H], F32R, kind="ExternalInput").ap()
    wk_d = nc.dram_tensor("wk", [EA, H], F32R, kind="ExternalInput").ap()
    watt_d = nc.dram_tensor("watt", [128, DT, G], F32, kind="ExternalInput").ap()
    wout_d = nc.dram_tensor("wout", [G * H, N_OUT], F32R, kind="ExternalInput").ap()
    bout_d = nc.dram_tensor("bout", [BL, N_OUT], F32, kind="ExternalInput").ap()
    glovet_d = nc.dram_tensor("glovet", [N_OUT, N_ANS], F32R,
                              kind="ExternalInput").ap()
    ident_d = nc.dram_tensor("ident", [128, 128], F32R, kind="ExternalInput").ap()
    ones_d = nc.dram_tensor("ones_col", [128, 1], F32R, kind="ExternalInput").ap()
    out_d = nc.dram_tensor("out", [BL, N_ANS], F32, kind="ExternalOutput").ap()
    warm_d = nc.dram_tensor("warm", [1, 128], F32, kind="ExternalOutput").ap()

    with tile.TileContext(nc) as tc:
        import contextlib

        with contextlib.ExitStack() as ctx:
            consts = ctx.enter_context(tc.tile_pool(name="consts", bufs=1))
            xrow_p = ctx.enter_context(tc.tile_pool(name="xrow", bufs=4))
            xkt_p = ctx.enter_context(tc.tile_pool(name="xkt", bufs=2))
            hkt_p = ctx.enter_context(tc.tile_pool(name="hkt", bufs=2))
            hk_p = ctx.enter_context(tc.tile_pool(name="hk", bufs=2))
            hqw_p = ctx.enter_context(tc.tile_pool(name="hqw", bufs=2))
            et_p = ctx.enter_context(tc.tile_pool(name="et", bufs=2))
            v_p = ctx.enter_context(tc.tile_pool(name="v", bufs=2))
            zz_p = ctx.enter_context(tc.tile_pool(name="zz", bufs=2))
            zn_p = ctx.enter_context(tc.tile_pool(name="zn", bufs=2))
            wout_p = ctx.enter_context(tc.tile_pool(name="wout", bufs=24))
            glove_p = ctx.enter_context(tc.tile_pool(name="glove", bufs=4))
            mm_p = ctx.enter_context(tc.tile_pool(name="mm", bufs=4, space="PSUM"))
            lg_p = ctx.enter_context(tc.tile_pool(name="lg", bufs=2, space="PSUM"))
            up_p = ctx.enter_context(tc.tile_pool(name="up", bufs=2, space="PSUM"))

            # ---- constants into SBUF ----
            ident = consts.tile([128, 128], F32R, tag="ident")
            nc.sync.dma_start(ident[:], ident_d)
            # PE warm-up: ~5us of back-to-back matmuls on the identity while
            # the initial DMAs stream in, so HAM reaches K=8/8 before real work
            wps = mm_p.tile([128, 512], F32, tag="mm")
            for i in range(48):
                nc.tensor.matmul(wps[:, :128], lhsT=ident[:], rhs=ident[:],
                                 start=True, stop=True)
            warm_sb = consts.tile([1, 128], F32, tag="warm")
            nc.vector.tensor_copy(warm_sb[:], wps[:1, :128])
            nc.sync.dma_start(warm_d, warm_sb[:])

            idxq_sb = consts.tile([128, TQ_TILES], I32, tag="idxq")
            nc.sync.dma_start(idxq_sb[:], idxq_d)
            idxk_sb = consts.tile([128, TK_TILES], I32, tag="idxk")
            nc.sync.dma_start(idxk_sb[:], idxk_d)
            wq_sb = consts.tile([128, 3, H], F32R, tag="wq")
            wk_sb = consts.tile([128, 3, H], F32R, tag="wk")
            for c, rows in enumerate(E_CHUNKS):
                nc.sync.dma_start(wq_sb[:rows, c, :], wq_d[c * 128 : c * 128 + rows])
                nc.sync.dma_start(wk_sb[:rows, c, :], wk_d[c * 128 : c * 128 + rows])
            watt_sb = consts.tile([128, DT, G], F32, tag="watt")
            nc.sync.dma_start(watt_sb[:], watt_d)
            bout_sb = consts.tile([BL, N_OUT], F32, tag="bout")
            nc.sync.dma_start(bout_sb[:], bout_d)
            ones_sb = consts.tile([128, 1], F32R, tag="ones")
            nc.sync.dma_start(ones_sb[:], ones_d)

            # ---- early-issued weight streams for phases F/G ----
            wout_tiles = []
            for t in range(NWOUT):
                wtile = wout_p.tile([128, N_OUT], F32R, tag="wout")
                nc.sync.dma_start(wtile[:], wout_d[t * 128 : (t + 1) * 128, :])
                wout_tiles.append(wtile)
            glove_tiles = []
            for a in range(NA_CH):
                gt = glove_p.tile([128, 3, NA_W], F32R, tag="glove")
                for c, rows in enumerate(N_CHUNKS):
                    nc.sync.dma_start(
                        gt[:rows, c, :],
                        glovet_d[c * 128 : c * 128 + rows,
                                 a * NA_W : (a + 1) * NA_W],
                    )
                glove_tiles.append(gt)

            def gather_transpose(idx_sb, col, dst, dst_col):
                """Gather 128 emb rows (token tile) and write transpose into
                dst[:, c, dst_col*128:...] per E-chunk c. The emb ones column
                (E index 300) lands at partition 44 of chunk 2."""
                xrow = xrow_p.tile([128, EA], F32R, tag="xrow")
                nc.gpsimd.indirect_dma_start(
                    out=xrow[:],
                    out_offset=None,
                    in_=emb_d,
                    in_offset=bass.IndirectOffsetOnAxis(
                        ap=idx_sb[:, col : col + 1], axis=0
                    ),
                )
                for c, rows in enumerate(E_CHUNKS):
                    ps = mm_p.tile([128, 512], F32R, tag="mm")
                    nc.tensor.transpose(
                        ps[:rows, :128], xrow[:, c * 128 : c * 128 + rows], ident[:]
                    )
                    nc.any.tensor_copy(
                        out=dst[:rows, c, dst_col * 128 : (dst_col + 1) * 128],
                        in_=ps[:rows, :128],
                    )

            # ---- phase B: gather+transpose Xq -> xqT [128, 3, TQ] ----
            xqT = consts.tile([128, 3, TQ], F32R, tag="xqT")
            for t in range(TQ_TILES):
                gather_transpose(idxq_sb, t, xqT, t)

            # ---- phase C: hqT [128, DT, TQ] (fp32; only DVE consumes it) ----
            hqT = consts.tile([128, DT, TQ], F32, tag="hqT")
            for m in range(DT):
                ps = mm_p.tile([128, 512], F32, tag="mm")
                for c, rows in enumerate(E_CHUNKS):
                    nc.tensor.matmul(
                        ps[:, :TQ],
                        lhsT=wq_sb[:rows, c, m * 128 : (m + 1) * 128],
                        rhs=xqT[:rows, c, :],
                        start=(c == 0),
                        stop=(c == 2),
                    )
                nc.vector.tensor_copy(hqT[:, m, :], ps[:, :TQ])

            poT = consts.tile([128, DT, G, BL], F32R, tag="poT")

            # ---- phase D: attention, two samples per D2 batch ----
            for p in range(BL // 2):
                # D1: gather + transpose K tokens for samples 2p, 2p+1
                xkT = xkt_p.tile([128, 3, 512], F32R, tag="xkT")
                for t in range(4):
                    gather_transpose(idxk_sb, p * 4 + t, xkT, t)

                # D2: hkT for the pair [128, DT, 512]
                hkT = hkt_p.tile([128, DT, 512], F32R, tag="hkT")
                for m in range(DT):
                    ps = mm_p.tile([128, 512], F32, tag="mm")
                    for c, rows in enumerate(E_CHUNKS):
                        nc.tensor.matmul(
                            ps[:],
                            lhsT=wk_sb[:rows, c, m * 128 : (m + 1) * 128],
                            rhs=xkT[:rows, c, :],
                            start=(c == 0),
                            stop=(c == 2),
                        )
                    nc.any.tensor_copy(out=hkT[:, m, :], in_=ps[:])

                for bi in range(2):
                    b = p * 2 + bi

                    # D3: hk_b [128, 2, H] (token-partition layout)
                    hk = hk_p.tile([128, 2, H], F32R, tag="hk")
                    for t in range(2):
                        for nchunk in range(2):
                            ps = mm_p.tile([128, 512], F32, tag="mm")
                            for c, rows in enumerate(E_CHUNKS):
                                nc.tensor.matmul(
                                    ps[:],
                                    lhsT=xkT[
                                        :rows, c,
                                        (bi * 2 + t) * 128 : (bi * 2 + t + 1) * 128,
                                    ],
                                    rhs=wk_sb[
                                        :rows, c, nchunk * 512 : (nchunk + 1) * 512
                                    ],
                                    start=(c == 0),
                                    stop=(c == 2),
                                )
                            nc.any.tensor_copy(
                                out=hk[:, t, nchunk * 512 : (nchunk + 1) * 512],
                                in_=ps[:],
                            )

                    # D4: hqw [128, DT, G, NQ] = hqT(b) * watt (bcast over g)
                    hqw = hqw_p.tile([128, DT, G, NQ], F32R, tag="hqw")
                    nc.vector.tensor_tensor(
                        out=hqw[:],
                        in0=hqT[:, :, None, b * NQ : (b + 1) * NQ].to_broadcast(
                            [128, DT, G, NQ]
                        ),
                        in1=watt_sb[:, :, :, None].to_broadcast([128, DT, G, NQ]),
                        op=OP.mult,
                    )

                    # D5: logits.T [k, (g,q)] in PSUM: [128, 2, 256]
                    ps_l = lg_p.tile([128, 512], F32, tag="lg")
                    for kt in range(2):
                        for c in range(DT):
                            nc.tensor.matmul(
                                ps_l[:, kt * 256 : (kt + 1) * 256],
                                lhsT=hkT[
                                    :, c,
                                    bi * 256 + kt * 128 : bi * 256 + (kt + 1) * 128,
                                ],
                                rhs=hqw[:, c],
                                start=(c == 0),
                                stop=(c == DT - 1),
                            )

                    # D6: E = exp(logits), per-(g)-block row sums zz
                    et = et_p.tile([128, 2, G * NQ], F32R, tag="et")
                    zz = zz_p.tile([128, 2, G], F32R, tag="zz")
                    for kt in range(2):
                        nc.scalar.activation(
                            out=et[:, kt, :],
                            in_=ps_l[:, kt * 256 : (kt + 1) * 256],
                            func=AF.Exp,
                        )
                        with nc.allow_low_precision(reason="fp32r round of f32 sum"):
                            nc.vector.tensor_reduce(
                                out=zz[:, kt, :],
                                in_=et[:, kt].rearrange("p (g q) -> p g q", g=G),
                                axis=AX.X,
                                op=OP.add,
                            )

                    # D7: Z_g = sum over k-partitions; zinv_b = 1/Z broadcast
                    ps_z = mm_p.tile([128, 512], F32, tag="mm")
                    for kt in range(2):
                        nc.tensor.matmul(
                            ps_z[:1, :G],
                            lhsT=ones_sb[:],
                            rhs=zz[:, kt, :],
                            start=(kt == 0),
                            stop=(kt == 1),
                        )
                    zinv = zn_p.tile([1, G], F32, tag="zinv")
                    nc.vector.reciprocal(zinv[:1, :], ps_z[:1, :G])
                    zbro = zn_p.tile([128, G], F32, tag="zbro")
                    nc.gpsimd.partition_broadcast(zbro[:], zinv[:1, :], channels=128)

                    # D8: u.T, v, pooled partial sums; 2 d-tiles per PSUM tile
                    for mp in range(4):
                        ps_u = up_p.tile([128, 512], F32, tag="up")
                        for mi in range(2):
                            m = mp * 2 + mi
                            for kt in range(2):
                                nc.tensor.matmul(
                                    ps_u[:, mi * 256 : (mi + 1) * 256],
                                    lhsT=hk[:, kt, m * 128 : (m + 1) * 128],
                                    rhs=et[:, kt, :],
                                    start=(kt == 0),
                                    stop=(kt == 1),
                                )
                        v = v_p.tile([128, 2, G, NQ], F32, tag="v")
                        nc.vector.tensor_tensor(
                            out=v[:],
                            in0=ps_u[:].rearrange("p (m g q) -> p m g q", m=2, g=G),
                            in1=hqT[
                                :, mp * 2 : mp * 2 + 2, None, b * NQ : (b + 1) * NQ
                            ].to_broadcast([128, 2, G, NQ]),
                            op=OP.mult,
                        )
                        vr = v_p.tile([128, 2, G], F32, tag="vr")
                        nc.vector.tensor_reduce(
                            out=vr[:], in_=v[:], axis=AX.X, op=OP.add
                        )
                        with nc.allow_low_precision(reason="fp32r round"):
                            nc.vector.tensor_tensor(
                                out=poT[:, mp * 2 : mp * 2 + 2, :, b],
                                in0=vr[:],
                                in1=zbro[:, None, :].to_broadcast([128, 2, G]),
                                op=OP.mult,
                            )

            # ---- phase F: out [8, 300] = pooled_flat @ Wout + bout ----
            ps_o = mm_p.tile([128, 512], F32, tag="mm")
            for g in range(G):
                for m in range(DT):
                    t = g * DT + m
                    nc.tensor.matmul(
                        ps_o[:BL, :N_OUT],
                        lhsT=poT[:, m, g, :],
                        rhs=wout_tiles[t][:],
                        start=(t == 0),
                        stop=(t == NWOUT - 1),
                    )
            out_sb = consts.tile([BL, N_OUT], F32R, tag="out_sb")
            nc.vector.tensor_tensor(
                out=out_sb[:], in0=ps_o[:BL, :N_OUT], in1=bout_sb[:], op=OP.add
            )

            # ---- phase G: sim + log_softmax ----
            outT = consts.tile([128, 3, BL], F32R, tag="outT")
            for c, rows in enumerate(N_CHUNKS):
                ps = mm_p.tile([128, 512], F32R, tag="mm")
                nc.tensor.transpose(
                    ps[:rows, :BL],
                    out_sb[:, c * 128 : c * 128 + rows],
                    ident[:BL, :BL],
                )
                nc.vector.tensor_copy(outT[:rows, c, :], ps[:rows, :BL])

            sim_sb = consts.tile([BL, N_ANS], F32, tag="sim_sb")
            esc = consts.tile([BL, NA_W], F32, tag="esc")
            mx8 = consts.tile([BL, NA_CH], F32, tag="mx8")
            zs8 = consts.tile([BL, NA_CH], F32, tag="zs8")
            mx = consts.tile([BL, 1], F32, tag="mx")
            nmx = consts.tile([BL, 1], F32, tag="nmx")
            zs = consts.tile([BL, 1], F32, tag="zs")
            lnz = consts.tile([BL, 1], F32, tag="lnz")
            for a in range(NA_CH):
                ps_s = mm_p.tile([128, 512], F32, tag="mm")
                for c, rows in enumerate(N_CHUNKS):
                    nc.tensor.matmul(
                        ps_s[:BL, :NA_W],
                        lhsT=outT[:rows, c, :],
                        rhs=glove_tiles[a][:rows, c, :],
                        start=(c == 0),
                        stop=(c == 2),
                    )
                nc.vector.tensor_reduce(
                    out=mx8[:, a : a + 1], in_=ps_s[:BL, :NA_W], axis=AX.X, op=OP.max
                )
                nc.vector.tensor_copy(sim_sb[:, a * NA_W : (a + 1) * NA_W],
                                      ps_s[:BL, :NA_W])
            nc.vector.tensor_reduce(out=mx[:], in_=mx8[:], axis=AX.X, op=OP.max)
            nc.vector.tensor_scalar_mul(nmx[:], mx[:], -1.0)
            for a in range(NA_CH):
                nc.scalar.activation(
                    out=esc[:],  # scratch, discarded
                    in_=sim_sb[:, a * NA_W : (a + 1) * NA_W],
                    func=AF.Exp,
                    bias=nmx[:],
                    accum_out=zs8[:, a : a + 1],
                )
            nc.vector.tensor_reduce(out=zs[:], in_=zs8[:], axis=AX.X, op=OP.add)
            nc.scalar.activation(out=lnz[:], in_=zs[:], func=AF.Ln)
            nc.vector.tensor_scalar(
                out=sim_sb[:],
                in0=sim_sb[:],
                scalar1=mx[:],
                scalar2=lnz[:],
                op0=OP.subtract,
                op1=OP.subtract,
            )
            nc.sync.dma_start(out_d, sim_sb[:])

    nc.compile()
    return nc


_NC = None


def _get_nc():
    global _NC
    if _NC is None:
        _NC = build_kernel()
    return _NC


def make_in_maps(inputs):
    he_q = np.asarray(inputs["he_ques"]).astype(np.int32)   # [64, 32]
    he_k = np.asarray(inputs["he_kg"]).astype(np.int32)     # [64, 256]
    emb0 = np.asarray(inputs["emb"], dtype=np.float32)
    emb = np.ones((VOCAB, EA), dtype=np.float32)            # ones col at E=300
    emb[:, :E] = emb0
    wq = np.concatenate(
        [np.asarray(inputs["Wq"], np.float32),
         np.asarray(inputs["bq"], np.float32)[None, :]], axis=0)
    wk = np.concatenate(
        [np.asarray(inputs["Wk"], np.float32),
         np.asarray(inputs["bk"], np.float32)[None, :]], axis=0)
    watt = np.ascontiguousarray(
        np.asarray(inputs["Watt"], np.float32).reshape(DT, 128, G)
        .transpose(1, 0, 2))                                # [128, DT, G]
    wout = np.ascontiguousarray(np.asarray(inputs["Wout"], np.float32))
    bout = np.ascontiguousarray(
        np.broadcast_to(np.asarray(inputs["bout"], np.float32), (BL, N_OUT)))
    glovet = np.ascontiguousarray(
        np.asarray(inputs["glove_cands"], np.float32).T)    # [300, 4000]
    ident = np.eye(128, dtype=np.float32)

    in_maps = []
    for i in range(N_CORES):
        iq = he_q[i * BL : (i + 1) * BL].reshape(-1)        # [256]
        ik = he_k[i * BL : (i + 1) * BL].reshape(-1)        # [2048]
        in_maps.append({
            "emb": emb,
            "idx_q": np.ascontiguousarray(iq.reshape(TQ_TILES, 128).T),
            "idx_k": np.ascontiguousarray(ik.reshape(TK_TILES, 128).T),
            "wq": wq,
            "wk": wk,
            "watt": watt,
            "wout": wout,
            "bout": bout,
            "glovet": glovet,
            "ident": ident,
            "ones_col": np.ones((128, 1), dtype=np.float32),
        })
    return in_maps


def kernel(**inputs) -> np.ndarray:
    nc = _get_nc()
    in_maps = make_in_maps(inputs)
    res = run_bass_kernel_spmd(nc, in_maps, list(range(N_CORES)))
    return np.concatenate([res.results[i]["out"] for i in range(N_CORES)], axis=0)
